# revision 6
# baseline (speedup 1.0000x reference)
"""Blocksparse matmul SSS (checkerboard layouts) on 8 trn2 NeuronCores.

Structure: BATCH=8 batches, 32x32 block grid, 128x128 fp32 blocks.
layout_x[r,k] = (r+k) even, layout_y[k,c] = (k+c) even, layout_o[r,c] = (r+c) even.
Every batch has 512 nnz blocks per tensor, stored contiguously (batch-major),
so sharding = one batch per core.

Within a batch, the checkerboard factorizes into TWO dense 2048^3 matmuls
(one per parity p of the output row-block index r):
  A_p[r', i] = x[(2r'+p)*16 + i]          (16x16 blocks, [m,k] layout)
  B_p[i, j]  = y[(2i+p)*16 + j]           (16x16 blocks, [k,c] layout)
  C_p[r', j] = out[(2r'+p)*16 + j] = sum_i A_p[r',i] @ B_p[i,j]

The PE contracts over the partition dim of both operands, so A blocks are
fed pre-transposed ([k,m]); the transpose is done on the host during
sharding (x.transpose(0,2,1) per block).

Device schedule per core: for each parity, keep all of B_p resident in
SBUF ([128, 32768] = 128KB/partition), stream A row-strips (1MB each), and
accumulate each output strip in 4 PSUM banks (N=512 fp32 per matmul,
16-step K accumulation).
"""

import os

os.environ.setdefault("MYCRO_LOCAL_CACHE", "1")

import numpy as np

import concourse.bacc as bacc
import concourse.bass as bass
import concourse.mybir as mybir
from concourse import tile
from concourse.bass_utils import run_bass_kernel_spmd

BS = 128          # sparsity block size
N_CORES = 8

# Populated by kernel() so a harness wrapper can read profiling info.
LAST_RESULTS = None


def build_program(G=32, n_cores=N_CORES):
    """Build the SPMD Bass program for one core (= one batch) of a
    G x G checkerboard block grid."""
    H = G // 2                 # nnz blocks per block-row
    NTOT = G * H               # nnz blocks per core per tensor
    NFREE = H * BS             # free width of a B/C strip
    NMM = min(512, NFREE)      # moving free dim per matmul (fp32 max 512)
    NJQ = NFREE // NMM         # psum groups per output strip
    f32 = mybir.dt.float32

    nc = bacc.Bacc("TRN2", target_bir_lowering=False, debug=False,
                   num_devices=n_cores)

    xt = nc.dram_tensor("xt", [NTOT, BS, BS], f32, kind="ExternalInput").ap()
    y = nc.dram_tensor("y", [NTOT, BS, BS], f32, kind="ExternalInput").ap()
    out = nc.dram_tensor("out", [NTOT, BS, BS], f32, kind="ExternalOutput").ap()

    with tile.TileContext(nc) as tc:
        with (
            tc.tile_pool(name="bpool", bufs=1) as bpool,
            tc.tile_pool(name="apool", bufs=3) as apool,
            tc.tile_pool(name="cpool", bufs=3) as cpool,
            tc.tile_pool(name="psum", bufs=2, space=bass.MemorySpace.PSUM) as pp,
        ):
            for p in range(2):
                btile = bpool.tile([BS, H * NFREE], f32, tag="B")
                for i in range(H):
                    k = 2 * i + p
                    nc.sync.dma_start(
                        out=btile[:, i * NFREE:(i + 1) * NFREE]
                        .rearrange("k (j c) -> k j c", j=H),
                        in_=y[k * H:(k + 1) * H].rearrange("j k c -> k j c"),
                    )
                for rp in range(H):
                    r = 2 * rp + p
                    atile = apool.tile([BS, H * BS], f32, tag="A")
                    nc.sync.dma_start(
                        out=atile[:].rearrange("k (i m) -> k i m", i=H),
                        in_=xt[r * H:(r + 1) * H].rearrange("i k m -> k i m"),
                    )
                    ctile = cpool.tile([BS, NFREE], f32, tag="C")
                    ptiles = [pp.tile([BS, NMM], f32, tag=f"ps{jq}",
                                      name=f"ps{jq}")
                              for jq in range(NJQ)]
                    for i in range(H):
                        lhsT = atile[:, i * BS:(i + 1) * BS]
                        for jq in range(NJQ):
                            nc.tensor.matmul(
                                ptiles[jq][:],
                                lhsT,
                                btile[:, i * NFREE + jq * NMM:
                                      i * NFREE + (jq + 1) * NMM],
                                start=(i == 0),
                                stop=(i == H - 1),
                            )
                    for jq in range(NJQ):
                        nc.vector.tensor_copy(
                            ctile[:, jq * NMM:(jq + 1) * NMM], ptiles[jq][:])
                    nc.sync.dma_start(
                        out=out[r * H:(r + 1) * H].rearrange("j m c -> m j c"),
                        in_=ctile[:].rearrange("m (j c) -> m j c", j=H),
                    )
    nc.compile()
    return nc


_PROGRAM = None


def _get_program():
    global _PROGRAM
    if _PROGRAM is None:
        _PROGRAM = build_program()
    return _PROGRAM


def make_in_maps(x, y):
    x = np.asarray(x, dtype=np.float32)
    y = np.asarray(y, dtype=np.float32)
    nb = x.shape[0] // N_CORES  # 512 blocks per core
    in_maps = []
    for b in range(N_CORES):
        xs = x[b * nb:(b + 1) * nb]
        in_maps.append({
            "xt": np.ascontiguousarray(xs.transpose(0, 2, 1)),
            "y": np.ascontiguousarray(y[b * nb:(b + 1) * nb]),
        })
    return in_maps


def kernel(x, y, sparsity_layout_x=None, sparsity_layout_y=None,
           sparsity_layout_output=None, o_n_sparse_blocks=None, **_kw):
    global LAST_RESULTS
    in_maps = make_in_maps(x, y)
    nc = _get_program()
    res = run_bass_kernel_spmd(nc, in_maps, list(range(N_CORES)))
    LAST_RESULTS = res
    return np.concatenate([res.results[b]["out"] for b in range(N_CORES)],
                          axis=0)


# revision 7
# speedup vs baseline: 3.2768x; 3.2768x over previous
"""Blocksparse matmul SSS (checkerboard layouts) on 8 trn2 NeuronCores.

Structure: BATCH=8 batches, 32x32 block grid, 128x128 fp32 blocks.
layout_x[r,k] = (r+k) even, layout_y[k,c] = (k+c) even, layout_o[r,c] = (r+c) even.
Every batch has 512 nnz blocks per tensor, stored contiguously (batch-major),
so sharding = one batch per core.

Within a batch, the checkerboard factorizes into TWO dense 2048^3 matmuls
(one per parity p of the output row-block index r):
  A_p[r', i] = x[(2r'+p)*16 + i]          (16x16 blocks, [m,k] layout)
  B_p[i, j]  = y[(2i+p)*16 + j]           (16x16 blocks, [k,c] layout)
  C_p[r', j] = out[(2r'+p)*16 + j] = sum_i A_p[r',i] @ B_p[i,j]

The PE contracts over the partition dim of both operands, so A blocks are
fed pre-transposed ([k,m]); the transpose is done on the host during
sharding (x.transpose(0,2,1) per block).

Device schedule per core: for each parity, keep all of B_p resident in
SBUF, stream A row-strips, and accumulate each output strip in 4 PSUM
banks (N=512 per matmul, K accumulation across 16 blocks).

Precision modes:
  f32    - plain fp32 matmuls (2 HW passes per matmul, slow but exact)
  f32r   - relaxed fp32 (1.5 cyc/row), same data bits
  fp16x3 - x,y split into hi+lo fp16 on host; C ~= xh@yh + xh@yl + xl@yh
  bf16x3 - same with bfloat16
  fp16   - plain fp16 (fast, ~1e-4 rel err)
"""

import os

os.environ.setdefault("MYCRO_LOCAL_CACHE", "1")

import numpy as np

import concourse.bacc as bacc
import concourse.bass as bass
import concourse.mybir as mybir
from concourse import tile
from concourse.bass_utils import run_bass_kernel_spmd

BS = 128          # sparsity block size
N_CORES = 8
MODE = os.environ.get("BS_KERNEL_MODE", "f32")

# Populated by kernel() so a harness wrapper can read profiling info.
LAST_RESULTS = None


def _split_dtypes(mode):
    if mode in ("fp16x3", "fp16"):
        return mybir.dt.float16, np.float16
    if mode == "bf16x3":
        import ml_dtypes
        return mybir.dt.bfloat16, ml_dtypes.bfloat16
    raise ValueError(mode)


def build_program(G=32, n_cores=N_CORES, mode=MODE):
    """Build the SPMD Bass program for one core (= one batch) of a
    G x G checkerboard block grid."""
    H = G // 2                 # nnz blocks per block-row
    NTOT = G * H               # nnz blocks per core per tensor
    NFREE = H * BS             # free width of a B/C strip
    NMM = min(512, NFREE)      # moving free dim per matmul
    NJQ = NFREE // NMM         # psum groups per output strip
    f32 = mybir.dt.float32

    split = mode in ("fp16x3", "bf16x3")
    if mode == "f32":
        mmdt = f32
    elif mode == "f32r":
        mmdt = mybir.dt.float32r
    else:
        mmdt, _ = _split_dtypes(mode)

    nc = bacc.Bacc("TRN2", target_bir_lowering=False, debug=False,
                   num_devices=n_cores)

    if split:
        xth = nc.dram_tensor("xth", [NTOT, BS, BS], mmdt, kind="ExternalInput").ap()
        xtl = nc.dram_tensor("xtl", [NTOT, BS, BS], mmdt, kind="ExternalInput").ap()
        yh = nc.dram_tensor("yh", [NTOT, BS, BS], mmdt, kind="ExternalInput").ap()
        yl = nc.dram_tensor("yl", [NTOT, BS, BS], mmdt, kind="ExternalInput").ap()
    else:
        xth = nc.dram_tensor("xt", [NTOT, BS, BS], mmdt, kind="ExternalInput").ap()
        yh = nc.dram_tensor("y", [NTOT, BS, BS], mmdt, kind="ExternalInput").ap()
    out = nc.dram_tensor("out", [NTOT, BS, BS], f32, kind="ExternalOutput").ap()

    def load_strip(dst, src, lo, n):
        """DMA blocks src[lo:lo+n] ([n,128,128]) into dst ([128, n*128])."""
        nc.sync.dma_start(
            out=dst.rearrange("k (i m) -> k i m", i=n),
            in_=src[lo:lo + n].rearrange("i k m -> k i m"),
        )

    with tile.TileContext(nc) as tc:
        with (
            tc.tile_pool(name="bpool", bufs=1) as bpool,
            tc.tile_pool(name="apool", bufs=3) as apool,
            tc.tile_pool(name="cpool", bufs=3) as cpool,
            tc.tile_pool(name="psum", bufs=2, space=bass.MemorySpace.PSUM) as pp,
        ):
            for p in range(2):
                btile = bpool.tile([BS, H * NFREE], mmdt, tag="B", name="bh")
                btl = (bpool.tile([BS, H * NFREE], mmdt, tag="Bl", name="bl")
                       if split else None)
                for i in range(H):
                    k = 2 * i + p
                    load_strip(btile[:, i * NFREE:(i + 1) * NFREE], yh, k * H, H)
                    if split:
                        load_strip(btl[:, i * NFREE:(i + 1) * NFREE], yl, k * H, H)
                for rp in range(H):
                    r = 2 * rp + p
                    atile = apool.tile([BS, H * BS], mmdt, tag="A", name="ah")
                    load_strip(atile[:], xth, r * H, H)
                    if split:
                        atl = apool.tile([BS, H * BS], mmdt, tag="Al", name="al")
                        load_strip(atl[:], xtl, r * H, H)
                    ctile = cpool.tile([BS, NFREE], f32, tag="C", name="ct")
                    ptiles = [pp.tile([BS, NMM], f32, tag=f"ps{jq}",
                                      name=f"ps{jq}") for jq in range(NJQ)]
                    nterm = 3 if split else 1
                    for i in range(H):
                        for jq in range(NJQ):
                            for t in range(nterm):
                                lhsT = (atile if t < 2 else atl)[
                                    :, i * BS:(i + 1) * BS]
                                rsrc = btl if t == 1 else btile
                                nc.tensor.matmul(
                                    ptiles[jq][:],
                                    lhsT,
                                    rsrc[:, i * NFREE + jq * NMM:
                                         i * NFREE + (jq + 1) * NMM],
                                    start=(i == 0 and t == 0),
                                    stop=(i == H - 1 and t == nterm - 1),
                                )
                    for jq in range(NJQ):
                        nc.vector.tensor_copy(
                            ctile[:, jq * NMM:(jq + 1) * NMM], ptiles[jq][:])
                    nc.sync.dma_start(
                        out=out[r * H:(r + 1) * H].rearrange("j m c -> m j c"),
                        in_=ctile[:].rearrange("m (j c) -> m j c", j=H),
                    )
    nc.compile()
    return nc


_PROGRAM = None


def _get_program():
    global _PROGRAM
    if _PROGRAM is None:
        _PROGRAM = build_program()
    return _PROGRAM


def make_in_maps(x, y, mode=MODE):
    x = np.asarray(x, dtype=np.float32)
    y = np.asarray(y, dtype=np.float32)
    nb = x.shape[0] // N_CORES
    split = mode in ("fp16x3", "bf16x3")
    in_maps = []
    for b in range(N_CORES):
        xt = np.ascontiguousarray(
            x[b * nb:(b + 1) * nb].transpose(0, 2, 1))
        ys = np.ascontiguousarray(y[b * nb:(b + 1) * nb])
        if split:
            _, npdt = _split_dtypes(mode)
            xth = xt.astype(npdt)
            xtl = (xt - xth.astype(np.float32)).astype(npdt)
            yhh = ys.astype(npdt)
            yll = (ys - yhh.astype(np.float32)).astype(npdt)
            in_maps.append({"xth": xth, "xtl": xtl, "yh": yhh, "yl": yll})
        elif mode == "fp16":
            _, npdt = _split_dtypes(mode)
            in_maps.append({"xt": xt.astype(npdt), "y": ys.astype(npdt)})
        else:
            in_maps.append({"xt": xt, "y": ys})
    return in_maps


def kernel(x, y, sparsity_layout_x=None, sparsity_layout_y=None,
           sparsity_layout_output=None, o_n_sparse_blocks=None, **_kw):
    global LAST_RESULTS
    in_maps = make_in_maps(x, y)
    nc = _get_program()
    res = run_bass_kernel_spmd(nc, in_maps, list(range(N_CORES)))
    LAST_RESULTS = res
    return np.concatenate([res.results[b]["out"] for b in range(N_CORES)],
                          axis=0)


# revision 10
# speedup vs baseline: 3.6820x; 1.1237x over previous
"""Blocksparse matmul SSS (checkerboard layouts) on 8 trn2 NeuronCores.

Structure: BATCH=8 batches, 32x32 block grid, 128x128 fp32 blocks.
layout_x[r,k] = (r+k) even, layout_y[k,c] = (k+c) even, layout_o[r,c] = (r+c) even.
Every batch has 512 nnz blocks per tensor, stored contiguously (batch-major),
so sharding = one batch per core.

Within a batch, the checkerboard factorizes into TWO dense 2048^3 matmuls
(one per parity p of the output row-block index r):
  A_p[r', i] = x[(2r'+p)*16 + i]          (16x16 blocks, [m,k] layout)
  B_p[i, j]  = y[(2i+p)*16 + j]           (16x16 blocks, [k,c] layout)
  C_p[r', j] = out[(2r'+p)*16 + j] = sum_i A_p[r',i] @ B_p[i,j]

The PE contracts over the partition dim of both operands, so A blocks are
fed pre-transposed ([k,m]); the transpose is done on the host during
sharding (x.transpose(0,2,1) per block).

Device schedule per core: for each parity, keep all of B_p resident in
SBUF, stream A row-strips, and accumulate each output strip in 4 PSUM
banks (N=512 per matmul, K accumulation across 16 blocks).

Precision modes:
  f32    - plain fp32 matmuls (2 HW passes per matmul, slow but exact)
  f32r   - relaxed fp32 (1.5 cyc/row), same data bits
  fp16x3 - x,y split into hi+lo fp16 on host; C ~= xh@yh + xh@yl + xl@yh
  bf16x3 - same with bfloat16
  fp16   - plain fp16 (fast, ~1e-4 rel err)
"""

import os

os.environ.setdefault("MYCRO_LOCAL_CACHE", "1")

import numpy as np

import concourse.bacc as bacc
import concourse.bass as bass
import concourse.mybir as mybir
from concourse import tile
from concourse.bass_utils import run_bass_kernel_spmd

BS = 128          # sparsity block size
N_CORES = 8
MODE = os.environ.get("BS_KERNEL_MODE", "f32")

# Populated by kernel() so a harness wrapper can read profiling info.
LAST_RESULTS = None


def _split_dtypes(mode):
    if mode in ("fp16x3", "fp16"):
        return mybir.dt.float16, np.float16
    if mode == "bf16x3":
        import ml_dtypes
        return mybir.dt.bfloat16, ml_dtypes.bfloat16
    raise ValueError(mode)


def build_program(G=32, n_cores=N_CORES, mode=MODE, nsplit=None):
    """Build the SPMD Bass program for one core (= one batch) of a
    G x G checkerboard block grid.

    nsplit: number of N-dimension sections per parity. Sections sized so
    the resident B tile fits 64KB/partition, letting the B pool
    double-buffer (bufs=2) and prefetch the next section/parity's B
    entirely under compute. Costs re-streaming the A strips once per
    section."""
    H = G // 2                 # nnz blocks per block-row
    NTOT = G * H               # nnz blocks per core per tensor
    NFREE = H * BS             # free width of a full B/C strip
    f32 = mybir.dt.float32

    split = mode in ("fp16x3", "bf16x3")
    if mode == "f32":
        mmdt = f32
    elif mode == "f32r":
        mmdt = mybir.dt.float32r
    else:
        mmdt, _ = _split_dtypes(mode)

    if nsplit is None:
        nsplit = {"f32": 2, "f32r": 2, "fp16": 1, "bf16x3": 2,
                  "fp16x3": 2}[mode]
    HS = H // nsplit           # c-blocks per section
    NW = HS * BS               # free width of a section strip
    NMM = min(512, NW)         # moving free dim per matmul
    NJQ = NW // NMM            # psum groups per section strip

    nc = bacc.Bacc("TRN2", target_bir_lowering=False, debug=False,
                   num_devices=n_cores)

    if split:
        xth = nc.dram_tensor("xth", [NTOT, BS, BS], mmdt, kind="ExternalInput").ap()
        xtl = nc.dram_tensor("xtl", [NTOT, BS, BS], mmdt, kind="ExternalInput").ap()
        yh = nc.dram_tensor("yh", [NTOT, BS, BS], mmdt, kind="ExternalInput").ap()
        yl = nc.dram_tensor("yl", [NTOT, BS, BS], mmdt, kind="ExternalInput").ap()
    else:
        xth = nc.dram_tensor("xt", [NTOT, BS, BS], mmdt, kind="ExternalInput").ap()
        yh = nc.dram_tensor("y", [NTOT, BS, BS], mmdt, kind="ExternalInput").ap()
    out = nc.dram_tensor("out", [NTOT, BS, BS], f32, kind="ExternalOutput").ap()

    def load_strip(dst, src, lo, n):
        """DMA blocks src[lo:lo+n] ([n,128,128]) into dst ([128, n*128])."""
        nc.sync.dma_start(
            out=dst.rearrange("k (i m) -> k i m", i=n),
            in_=src[lo:lo + n].rearrange("i k m -> k i m"),
        )

    with tile.TileContext(nc) as tc:
        with (
            tc.tile_pool(name="bpool", bufs=2) as bpool,
            tc.tile_pool(name="apool", bufs=3) as apool,
            tc.tile_pool(name="cpool", bufs=3) as cpool,
            tc.tile_pool(name="psum", bufs=max(1, 8 // NJQ),
                         space=bass.MemorySpace.PSUM) as pp,
        ):
            for p in range(2):
                for ns in range(nsplit):
                    j0 = ns * HS       # first c-block of this section
                    btile = bpool.tile([BS, H * NW], mmdt, tag="B", name="bh")
                    btl = (bpool.tile([BS, H * NW], mmdt, tag="Bl", name="bl")
                           if split else None)
                    for i in range(H):
                        k = 2 * i + p
                        load_strip(btile[:, i * NW:(i + 1) * NW],
                                   yh, k * H + j0, HS)
                        if split:
                            load_strip(btl[:, i * NW:(i + 1) * NW],
                                       yl, k * H + j0, HS)
                    for rp in range(H):
                        r = 2 * rp + p
                        atile = apool.tile([BS, H * BS], mmdt, tag="A",
                                           name="ah")
                        load_strip(atile[:], xth, r * H, H)
                        if split:
                            atl = apool.tile([BS, H * BS], mmdt, tag="Al",
                                             name="al")
                            load_strip(atl[:], xtl, r * H, H)
                        ctile = cpool.tile([BS, NW], f32, tag="C", name="ct")
                        ptiles = [pp.tile([BS, NMM], f32, tag=f"ps{jq}",
                                          name=f"ps{jq}") for jq in range(NJQ)]
                        nterm = 3 if split else 1
                        for i in range(H):
                            for jq in range(NJQ):
                                for t in range(nterm):
                                    lhsT = (atile if t < 2 else atl)[
                                        :, i * BS:(i + 1) * BS]
                                    rsrc = btl if t == 1 else btile
                                    nc.tensor.matmul(
                                        ptiles[jq][:],
                                        lhsT,
                                        rsrc[:, i * NW + jq * NMM:
                                             i * NW + (jq + 1) * NMM],
                                        start=(i == 0 and t == 0),
                                        stop=(i == H - 1 and t == nterm - 1),
                                    )
                        for jq in range(NJQ):
                            nc.vector.tensor_copy(
                                ctile[:, jq * NMM:(jq + 1) * NMM],
                                ptiles[jq][:])
                        nc.sync.dma_start(
                            out=out[r * H + j0:r * H + j0 + HS]
                            .rearrange("j m c -> m j c"),
                            in_=ctile[:].rearrange("m (j c) -> m j c", j=HS),
                        )
    nc.compile()
    return nc


_PROGRAM = None


def _get_program():
    global _PROGRAM
    if _PROGRAM is None:
        _PROGRAM = build_program()
    return _PROGRAM


def make_in_maps(x, y, mode=MODE):
    x = np.asarray(x, dtype=np.float32)
    y = np.asarray(y, dtype=np.float32)
    nb = x.shape[0] // N_CORES
    split = mode in ("fp16x3", "bf16x3")
    in_maps = []
    for b in range(N_CORES):
        xt = np.ascontiguousarray(
            x[b * nb:(b + 1) * nb].transpose(0, 2, 1))
        ys = np.ascontiguousarray(y[b * nb:(b + 1) * nb])
        if split:
            _, npdt = _split_dtypes(mode)
            xth = xt.astype(npdt)
            xtl = (xt - xth.astype(np.float32)).astype(npdt)
            yhh = ys.astype(npdt)
            yll = (ys - yhh.astype(np.float32)).astype(npdt)
            in_maps.append({"xth": xth, "xtl": xtl, "yh": yhh, "yl": yll})
        elif mode == "fp16":
            _, npdt = _split_dtypes(mode)
            in_maps.append({"xt": xt.astype(npdt), "y": ys.astype(npdt)})
        else:
            in_maps.append({"xt": xt, "y": ys})
    return in_maps


def kernel(x, y, sparsity_layout_x=None, sparsity_layout_y=None,
           sparsity_layout_output=None, o_n_sparse_blocks=None, **_kw):
    global LAST_RESULTS
    in_maps = make_in_maps(x, y)
    nc = _get_program()
    res = run_bass_kernel_spmd(nc, in_maps, list(range(N_CORES)))
    LAST_RESULTS = res
    return np.concatenate([res.results[b]["out"] for b in range(N_CORES)],
                          axis=0)
